# revision 33
# baseline (speedup 1.0000x reference)
"""BlockLinear kernel for Trainium2 (8 NeuronCores, SPMD).

y[b, g*512+o] = sum_i x[b, g*512+i] * W[g, o, i] + bias[g, o]

Sharding: one block g per core (expert parallelism). Each core computes
y_g = x_g @ W_g^T + b_g with x_g = x[:, g*512:(g+1)*512].

Per-core device kernel (~128us HW; PE floor for this decomposition is
511 matmuls x 216ns = 110.4us + ~7.2us fixed NEFF prologue + ramp+tail):
  - x is fed in fp8-e3m4 (PE runs mixed fp8xfp16 at full rate; halves
    x DMA bytes; end-to-end rel err 1.14e-2 vs the 2e-2 gate), in a
    per-group-packed partition-major layout (one contiguous ~6 KB
    descriptor per partition per group); y is stored partition-major
    (contiguous per-partition descriptors) and the host un-permutes.
  - Ring assignment matches stream demand to measured ring throughput:
    the scalar HWDGE ring carries the x stream (77 GB/s sustained), the
    sync HWDGE ring carries bias + all y-out (152 GB/s, chunked every 4
    subtiles so output issue isn't end-loaded); W is k-split across BOTH
    rings ahead of everything (each ring's early throughput is only
    ~100 KB/us while the DGE pipeline fills).  Engine DGE queues are
    FIFO, so y's DVE-gated DMAs stay off x's ring.
  - Batch is processed in groups (small head ramp, 1536-row body,
    384/256/128 tail so the final output DMA chain is short).  Per
    128-row subtile: 4 accumulating matmuls into a PSUM bank (8-deep
    pool), DVE adds bias while copying PSUM->SBUF (casting to fp16).
  - 10 warmup fp16 matmuls on a memset tile keep the PE busy until the
    first x/W tiles land, and get the HAM clock-gate to 2.4 GHz
    (~3.4us of sustained PE activity) before the real stream starts.
  - Pitfall: untagged tiles in one pool share a slot tag and the Tile
    scheduler serializes their DMAs — all const tiles carry distinct
    tags.

Schemes: "f8x" (x fp8-e3m4, W fp16; rel err ~1.1e-2), "f16" (x,W fp16;
rel err ~3e-4, ~2-4us slower from doubled x traffic).
"""

import numpy as np

import concourse.bass as bass
import concourse.mybir as mybir
import concourse.tile as tile
from concourse import bacc
from concourse.bass_utils import run_bass_kernel_spmd
from concourse.vector_clock import ScopedClock

F32 = mybir.dt.float32

NB, BIN, BOUT = 8, 512, 512
BATCH = 16384
NCORES = 8
P = 128
KT = BIN // P  # 4 k-tiles per block
NSUB = BATCH // P  # 128 output subtiles

SCHEME = "f8x"  # "f8x" | "f16"

_patched = False


def _patch_tile_drain():
    """Walrus in this container accepts only one sync-wait per InstDrain;
    split the tile-exit drain's waits across one drain instruction each."""
    global _patched
    if _patched:
        return
    _patched = True

    def _drain_and_barrier(self, tick_clock, wait_clock):
        nc = self.nc
        drain_inst = nc.sync.drain()
        wait_clock.add_sem_waits(
            drain_inst.ins, ScopedClock({None: tick_clock.global_clock})
        )
        si = drain_inst.ins.sync_info
        if si is not None and len(si.on_wait) > 1:
            waits = list(si.on_wait)
            updates = list(si.on_update)
            drain_inst.ins.sync_info = mybir.SyncInfo(
                on_wait=[waits[0]], on_update=updates
            )
            for w in waits[1:]:
                extra = nc.sync.drain()
                extra.ins.sync_info = mybir.SyncInfo(on_wait=[w], on_update=[])
        nc.all_engine_barrier()
        popped = nc._tile_sem_poison_stack.pop()
        assert popped is self._sem_poison
        # Skip Tile's exit-time sem clear + second barrier: walrus's
        # end-of-NEFF epilogue unconditionally zeroes every semaphore on
        # every engine, and nothing runs between the barrier above and
        # that epilogue. (Verified: repeated executions stay correct.)
        sems = list(self.sems.allocated().values())
        sem_nums = [s.num if hasattr(s, "num") else s for s in sems]
        nc._state.prepend_free_semaphores(sem_nums)
        for poison_set in nc._tile_sem_poison_stack:
            poison_set.update(sem_nums)

    tile.TileContext._drain_and_barrier = _drain_and_barrier


_nc_cache = {}


def _scheme_dtypes(scheme):
    if scheme == "f16":
        return mybir.dt.float16, np.float16
    elif scheme == "f8x":
        import ml_dtypes

        return mybir.dt.float8e3, ml_dtypes.float8_e3m4
    raise ValueError(scheme)


def _groups(body=1536):
    """Batch-row group sizes: geometric head ramp (matmuls start on the
    first small tile while DMA builds runway), fixed-size body, small
    final groups (the kernel tail only waits on small output DMAs)."""
    head = [256, 256, 512, 1024]
    tail = [384, 256, 128]
    mid = BATCH - sum(head) - sum(tail)
    sizes = head + [body] * (mid // body)
    rem = mid % body
    if rem:
        sizes.append(rem)
    sizes += tail
    assert sum(sizes) == BATCH, sizes
    return sizes


def _build(scheme=SCHEME, body=1536):
    key = (scheme, body)
    if key in _nc_cache:
        return _nc_cache[key]
    _patch_tile_drain()
    x_dt, _ = _scheme_dtypes(scheme)
    w_dt = mybir.dt.float16
    out_dt = mybir.dt.float16

    nc = bacc.Bacc(None, target_bir_lowering=False)
    # x: per-group-packed, partition-major: for each group g of rows
    # [r0,r1): x_dev[p, off_g + k*gsz + (b-r0)] = x_g[r0+b... ] — i.e.
    # per partition, KT contiguous chunks of gsz values.
    xP = nc.dram_tensor("xP", [P, KT * BATCH], x_dt, kind="ExternalInput")
    wT = nc.dram_tensor("wT", [BIN, BOUT], w_dt, kind="ExternalInput")
    bias = nc.dram_tensor("bias", [P, BOUT], F32, kind="ExternalInput")
    # y: partition-major [p, s*BOUT + o] for output row s*P + p.
    y = nc.dram_tensor("y", [P, NSUB * BOUT], out_dt, kind="ExternalOutput")

    groups = _groups(body)

    with tile.TileContext(nc) as tc:
        with (
            tc.tile_pool(name="const", bufs=1) as const,
            tc.tile_pool(name="xp", bufs=10 if scheme == "f8x" else 6) as xp,
            tc.tile_pool(name="yp", bufs=6) as yp,
            tc.tile_pool(name="ps", bufs=8, space="PSUM") as psp,
        ):
            # ---- emission order per engine is execution order ----
            # sync:   W per-k DMAs, bias, per-group y DMAs
            # scalar: the x stream (group 0 split per k-tile), nothing else
            # tensor: warmup matmuls, then the real matmul stream
            # vector: per-subtile bias-add + PSUM->SBUF copy
            # gpsimd: one small memset for the warmup operands

            # NB: distinct tags — untagged tiles in one pool share a single
            # slot tag and the scheduler serializes their issue.
            # W is split across BOTH rings (each ring's early throughput is
            # only ~100 KB/us) so all four k-slices land ~2.5us sooner.
            wk = const.tile([P, KT, BOUT], w_dt, tag="wk")
            for k in range(KT):
                eng = nc.sync if k % 2 == 0 else nc.scalar
                eng.dma_start(wk[:, k, :], wT[k * P : (k + 1) * P, :])
            bt = const.tile([P, BOUT], F32, tag="bt")
            nc.sync.dma_start(bt[:], bias[:])

            # PE warmup (fp16, 427ns each cold) until W/x0 land and the
            # HAM clock-gate has ramped.
            junk = const.tile([P, 640], mybir.dt.float16, tag="junk")
            nc.gpsimd.memset(junk[:], 0.0)
            warm_ps = psp.tile([P, BOUT], F32, tag="ps")
            for _ in range(10):
                nc.tensor.matmul(
                    warm_ps[:, :],
                    junk[:, :128],
                    junk[:, 128:640],
                    start=True,
                    stop=True,
                )

            row = 0
            off = 0
            YC = 4  # y-out chunk: DMA every 4 subtiles so output flows
            for gi, gsz in enumerate(groups):
                nsub = gsz // P
                xs = xp.tile([P, KT * gsz], x_dt, tag="xt")
                nc.scalar.dma_start(xs[:], xP[:, off : off + KT * gsz])
                off += KT * gsz
                yt = yp.tile([P, nsub * BOUT], out_dt, tag="yt")
                s0 = row // P
                c0 = 0
                last = gi == len(groups) - 1
                for ms in range(nsub):
                    ps = psp.tile([P, BOUT], F32, tag="ps")
                    if last and ms == nsub - 1:
                        # final subtile: split into two 256-col halves so
                        # the first half's DVE add + y DMA overlap the
                        # second half's matmuls — shortens the serial
                        # end-of-kernel chain by ~0.4us.
                        if ms > c0:
                            nc.sync.dma_start(
                                y[:, (s0 + c0) * BOUT : (s0 + ms) * BOUT],
                                yt[:, c0 * BOUT : ms * BOUT],
                            )
                        for h in range(2):
                            ho = h * (BOUT // 2)
                            for k in range(KT):
                                nc.tensor.matmul(
                                    ps[:, ho : ho + BOUT // 2],
                                    xs[:, k * gsz + ms * P : k * gsz + (ms + 1) * P],
                                    wk[:, k, ho : ho + BOUT // 2],
                                    start=(k == 0),
                                    stop=(k == KT - 1),
                                )
                            nc.vector.tensor_add(
                                out=yt[:, ms * BOUT + ho : ms * BOUT + ho + BOUT // 2],
                                in0=ps[:, ho : ho + BOUT // 2],
                                in1=bt[:, ho : ho + BOUT // 2],
                            )
                            nc.sync.dma_start(
                                y[
                                    :,
                                    (s0 + ms) * BOUT + ho : (s0 + ms) * BOUT
                                    + ho
                                    + BOUT // 2,
                                ],
                                yt[:, ms * BOUT + ho : ms * BOUT + ho + BOUT // 2],
                            )
                        c0 = ms + 1
                        continue
                    for k in range(KT):
                        nc.tensor.matmul(
                            ps[:],
                            xs[:, k * gsz + ms * P : k * gsz + (ms + 1) * P],
                            wk[:, k, :],
                            start=(k == 0),
                            stop=(k == KT - 1),
                        )
                    nc.vector.tensor_add(
                        out=yt[:, ms * BOUT : (ms + 1) * BOUT], in0=ps[:], in1=bt[:]
                    )
                    if ms + 1 - c0 == YC or ms + 1 == nsub:
                        nc.sync.dma_start(
                            y[:, (s0 + c0) * BOUT : (s0 + ms + 1) * BOUT],
                            yt[:, c0 * BOUT : (ms + 1) * BOUT],
                        )
                        c0 = ms + 1
                row += gsz
    nc.compile()
    _nc_cache[key] = nc
    return nc


def _pack_x(xT_g, groups, x_np):
    """[BIN, BATCH] -> per-group-packed partition-major [P, KT*BATCH]."""
    x3 = np.ascontiguousarray(xT_g.reshape(KT, P, BATCH).transpose(1, 0, 2))
    out = np.empty((P, KT * BATCH), dtype=x_np)
    off = 0
    r = 0
    for gsz in groups:
        out[:, off : off + KT * gsz] = x3[:, :, r : r + gsz].reshape(P, KT * gsz)
        off += KT * gsz
        r += gsz
    return out


LAST_RESULT = None


def kernel(x, W, b, trace=False, scheme=SCHEME, body=1536, trace_kwargs=None):
    global LAST_RESULT
    x = np.asarray(x, dtype=np.float32)
    W = np.asarray(W, dtype=np.float32)
    b = np.asarray(b, dtype=np.float32)

    _, x_np = _scheme_dtypes(scheme)
    nc = _build(scheme, body)
    groups = _groups(body)
    in_maps = []
    for g in range(NCORES):
        xT_g = x[:, g * BIN : (g + 1) * BIN].T.astype(x_np)
        xP_g = _pack_x(xT_g, groups, x_np)
        wT_g = np.ascontiguousarray(W[g].T.astype(np.float16))
        bias_g = np.ascontiguousarray(np.broadcast_to(b[g][None, :], (P, BOUT)))
        in_maps.append({"xP": xP_g, "wT": wT_g, "bias": bias_g})

    kwargs = dict(trace_kwargs or {})
    res = run_bass_kernel_spmd(nc, in_maps, list(range(NCORES)), trace=trace, **kwargs)
    LAST_RESULT = res

    out = np.empty((BATCH, NB * BOUT), dtype=np.float32)
    for g in range(NCORES):
        yp = res.results[g]["y"].reshape(P, NSUB, BOUT)
        out[:, g * BOUT : (g + 1) * BOUT] = (
            yp.transpose(1, 0, 2).reshape(BATCH, BOUT).astype(np.float32)
        )
    return out


# revision 34
# speedup vs baseline: 1.0286x; 1.0286x over previous
"""BlockLinear kernel for Trainium2 (8 NeuronCores, SPMD).

y[b, g*512+o] = sum_i x[b, g*512+i] * W[g, o, i] + bias[g, o]

Sharding: one block g per core (expert parallelism). Each core computes
y_g = x_g @ W_g^T + b_g with x_g = x[:, g*512:(g+1)*512].

Per-core device kernel (~128us HW; PE floor for this decomposition is
511 matmuls x 216ns = 110.4us + ~7.2us fixed NEFF prologue + ramp+tail):
  - x is fed in fp8-e3m4 (PE runs mixed fp8xfp16 at full rate; halves
    x DMA bytes; end-to-end rel err 1.14e-2 vs the 2e-2 gate), in a
    per-group-packed partition-major layout (one contiguous ~6 KB
    descriptor per partition per group); y is stored partition-major
    (contiguous per-partition descriptors) and the host un-permutes.
  - Ring assignment matches stream demand to measured ring throughput:
    the scalar HWDGE ring carries the x stream (77 GB/s sustained), the
    sync HWDGE ring carries bias + all y-out (152 GB/s, chunked every 4
    subtiles so output issue isn't end-loaded); W is k-split across BOTH
    rings ahead of everything (each ring's early throughput is only
    ~100 KB/us while the DGE pipeline fills).  Engine DGE queues are
    FIFO, so y's DVE-gated DMAs stay off x's ring.
  - Batch is processed in groups (small head ramp, 1536-row body,
    384/256/128 tail so the final output DMA chain is short).  Per
    128-row subtile: 4 accumulating matmuls into a PSUM bank (8-deep
    pool), DVE adds bias while copying PSUM->SBUF (casting to fp16).
  - 10 warmup fp16 matmuls on a memset tile keep the PE busy until the
    first x/W tiles land, and get the HAM clock-gate to 2.4 GHz
    (~3.4us of sustained PE activity) before the real stream starts.
  - Pitfall: untagged tiles in one pool share a slot tag and the Tile
    scheduler serializes their DMAs — all const tiles carry distinct
    tags.

Schemes: "f8x" (x fp8-e3m4, W fp16; rel err ~1.1e-2), "f16" (x,W fp16;
rel err ~3e-4, ~2-4us slower from doubled x traffic).
"""

import numpy as np

import concourse.bass as bass
import concourse.mybir as mybir
import concourse.tile as tile
from concourse import bacc
from concourse.bass_utils import run_bass_kernel_spmd
from concourse.vector_clock import ScopedClock

F32 = mybir.dt.float32

NB, BIN, BOUT = 8, 512, 512
BATCH = 16384
NCORES = 8
P = 128
KT = BIN // P  # 4 k-tiles per block
NSUB = BATCH // P  # 128 output subtiles

SCHEME = "f8x"  # "f8x" | "f16"

_patched = False


def _patch_tile_drain():
    """Walrus in this container accepts only one sync-wait per InstDrain;
    split the tile-exit drain's waits across one drain instruction each."""
    global _patched
    if _patched:
        return
    _patched = True

    def _drain_and_barrier(self, tick_clock, wait_clock):
        nc = self.nc
        drain_inst = nc.sync.drain()
        wait_clock.add_sem_waits(
            drain_inst.ins, ScopedClock({None: tick_clock.global_clock})
        )
        si = drain_inst.ins.sync_info
        if si is not None and len(si.on_wait) > 1:
            waits = list(si.on_wait)
            updates = list(si.on_update)
            drain_inst.ins.sync_info = mybir.SyncInfo(
                on_wait=[waits[0]], on_update=updates
            )
            for w in waits[1:]:
                extra = nc.sync.drain()
                extra.ins.sync_info = mybir.SyncInfo(on_wait=[w], on_update=[])
        nc.all_engine_barrier()
        popped = nc._tile_sem_poison_stack.pop()
        assert popped is self._sem_poison
        # Skip Tile's exit-time sem clear + second barrier: walrus's
        # end-of-NEFF epilogue unconditionally zeroes every semaphore on
        # every engine, and nothing runs between the barrier above and
        # that epilogue. (Verified: repeated executions stay correct.)
        sems = list(self.sems.allocated().values())
        sem_nums = [s.num if hasattr(s, "num") else s for s in sems]
        nc._state.prepend_free_semaphores(sem_nums)
        for poison_set in nc._tile_sem_poison_stack:
            poison_set.update(sem_nums)

    tile.TileContext._drain_and_barrier = _drain_and_barrier


_nc_cache = {}


def _scheme_dtypes(scheme):
    if scheme == "f16":
        return mybir.dt.float16, np.float16
    elif scheme == "f8x":
        import ml_dtypes

        return mybir.dt.float8e3, ml_dtypes.float8_e3m4
    raise ValueError(scheme)


def _groups(body=1536):
    """Batch-row group sizes: geometric head ramp (matmuls start on the
    first small tile while DMA builds runway), fixed-size body, small
    final groups (the kernel tail only waits on small output DMAs)."""
    head = [256, 256, 512, 1024]
    tail = [384, 256, 128]
    mid = BATCH - sum(head) - sum(tail)
    sizes = head + [body] * (mid // body)
    rem = mid % body
    if rem:
        sizes.append(rem)
    sizes += tail
    assert sum(sizes) == BATCH, sizes
    return sizes


def _build(scheme=SCHEME, body=1536):
    key = (scheme, body)
    if key in _nc_cache:
        return _nc_cache[key]
    _patch_tile_drain()
    x_dt, _ = _scheme_dtypes(scheme)
    w_dt = mybir.dt.float16
    out_dt = mybir.dt.float16

    nc = bacc.Bacc(None, target_bir_lowering=False)
    # x: per-group-packed, partition-major: for each group g of rows
    # [r0,r1): x_dev[p, off_g + k*gsz + (b-r0)] = x_g[r0+b... ] — i.e.
    # per partition, KT contiguous chunks of gsz values.
    xP = nc.dram_tensor("xP", [P, KT * BATCH], x_dt, kind="ExternalInput")
    wT = nc.dram_tensor("wT", [BIN, BOUT], w_dt, kind="ExternalInput")
    bias = nc.dram_tensor("bias", [P, BOUT], F32, kind="ExternalInput")
    # y: partition-major [p, s*BOUT + o] for output row s*P + p.
    y = nc.dram_tensor("y", [P, NSUB * BOUT], out_dt, kind="ExternalOutput")

    groups = _groups(body)

    with tile.TileContext(nc) as tc:
        with (
            tc.tile_pool(name="const", bufs=1) as const,
            tc.tile_pool(name="xp", bufs=10 if scheme == "f8x" else 6) as xp,
            tc.tile_pool(name="yp", bufs=6) as yp,
            tc.tile_pool(name="ps", bufs=8, space="PSUM") as psp,
        ):
            # ---- emission order per engine is execution order ----
            # sync:   W per-k DMAs, bias, per-group y DMAs
            # scalar: the x stream (group 0 split per k-tile), nothing else
            # tensor: warmup matmuls, then the real matmul stream
            # vector: per-subtile bias-add + PSUM->SBUF copy
            # gpsimd: one small memset for the warmup operands

            # NB: distinct tags — untagged tiles in one pool share a single
            # slot tag and the scheduler serializes their issue.
            # W is split across BOTH rings (each ring's early throughput is
            # only ~100 KB/us) so all four k-slices land ~2.5us sooner.
            wk = const.tile([P, KT, BOUT], w_dt, tag="wk")
            for k in range(KT):
                eng = nc.sync if k % 2 == 0 else nc.scalar
                eng.dma_start(wk[:, k, :], wT[k * P : (k + 1) * P, :])
            bt = const.tile([P, BOUT], F32, tag="bt")
            nc.sync.dma_start(bt[:], bias[:])

            # PE warmup (fp16, 427ns each cold) until W/x0 land and the
            # HAM clock-gate has ramped.
            junk = const.tile([P, 640], mybir.dt.float16, tag="junk")
            nc.gpsimd.memset(junk[:], 0.0)
            warm_ps = psp.tile([P, BOUT], F32, tag="ps")
            for _ in range(10):
                nc.tensor.matmul(
                    warm_ps[:, :],
                    junk[:, :128],
                    junk[:, 128:640],
                    start=True,
                    stop=True,
                )

            row = 0
            off = 0
            YC = 4  # y-out chunk: DMA every 4 subtiles so output flows
            for gi, gsz in enumerate(groups):
                nsub = gsz // P
                xs = xp.tile([P, KT * gsz], x_dt, tag="xt")
                nc.scalar.dma_start(xs[:], xP[:, off : off + KT * gsz])
                off += KT * gsz
                yt = yp.tile([P, nsub * BOUT], out_dt, tag="yt")
                s0 = row // P
                c0 = 0
                for ms in range(nsub):
                    ps = psp.tile([P, BOUT], F32, tag="ps")
                    for k in range(KT):
                        nc.tensor.matmul(
                            ps[:],
                            xs[:, k * gsz + ms * P : k * gsz + (ms + 1) * P],
                            wk[:, k, :],
                            start=(k == 0),
                            stop=(k == KT - 1),
                        )
                    nc.vector.tensor_add(
                        out=yt[:, ms * BOUT : (ms + 1) * BOUT], in0=ps[:], in1=bt[:]
                    )
                    if ms + 1 - c0 == YC or ms + 1 == nsub:
                        nc.sync.dma_start(
                            y[:, (s0 + c0) * BOUT : (s0 + ms + 1) * BOUT],
                            yt[:, c0 * BOUT : (ms + 1) * BOUT],
                        )
                        c0 = ms + 1
                row += gsz
    nc.compile()
    _nc_cache[key] = nc
    return nc


def _pack_x(xT_g, groups, x_np):
    """[BIN, BATCH] -> per-group-packed partition-major [P, KT*BATCH]."""
    x3 = np.ascontiguousarray(xT_g.reshape(KT, P, BATCH).transpose(1, 0, 2))
    out = np.empty((P, KT * BATCH), dtype=x_np)
    off = 0
    r = 0
    for gsz in groups:
        out[:, off : off + KT * gsz] = x3[:, :, r : r + gsz].reshape(P, KT * gsz)
        off += KT * gsz
        r += gsz
    return out


LAST_RESULT = None


def kernel(x, W, b, trace=False, scheme=SCHEME, body=1536, trace_kwargs=None):
    global LAST_RESULT
    x = np.asarray(x, dtype=np.float32)
    W = np.asarray(W, dtype=np.float32)
    b = np.asarray(b, dtype=np.float32)

    _, x_np = _scheme_dtypes(scheme)
    nc = _build(scheme, body)
    groups = _groups(body)
    in_maps = []
    for g in range(NCORES):
        xT_g = x[:, g * BIN : (g + 1) * BIN].T.astype(x_np)
        xP_g = _pack_x(xT_g, groups, x_np)
        wT_g = np.ascontiguousarray(W[g].T.astype(np.float16))
        bias_g = np.ascontiguousarray(np.broadcast_to(b[g][None, :], (P, BOUT)))
        in_maps.append({"xP": xP_g, "wT": wT_g, "bias": bias_g})

    kwargs = dict(trace_kwargs or {})
    res = run_bass_kernel_spmd(nc, in_maps, list(range(NCORES)), trace=trace, **kwargs)
    LAST_RESULT = res

    out = np.empty((BATCH, NB * BOUT), dtype=np.float32)
    for g in range(NCORES):
        yp = res.results[g]["y"].reshape(P, NSUB, BOUT)
        out[:, g * BOUT : (g + 1) * BOUT] = (
            yp.transpose(1, 0, 2).reshape(BATCH, BOUT).astype(np.float32)
        )
    return out
